# revision 3
# baseline (speedup 1.0000x reference)
"""Trainium2 Bass kernel for nn_Conv4d (K separate Conv3d layers folded into a
single conv3d with K*Co output channels + temporal accumulation).

Problem (hardcoded):
  x:      [B=2, Ci=8, T=16, D=40, H=40, W=40] f32
  weight: [K=3, Co=32, Ci=8, 3, 3, 3] f32
  bias:   [K=3, Co=32] f32
  out:    [B=2, Co=32, O=16, 40, 40, 40] f32
  out[b, co, o] = sum_k ( conv3d(x[b, :, o+k-1], weight[k], pad=1) + bias[k] )
  with out-of-range temporal frames skipped (zero contribution, incl. bias).

Sharding: data-parallel over the 32 B*T frames, 4 consecutive frames per core.
Each core computes conv3d y[j] = W * x[j] + bias for its 4 frames and
accumulates frame j's three k-blocks into output-frame partials
o = j-1, j, j+1 on-chip; partial sums are combined on the host (adjacent
cores/accumulators overlap at boundaries; host sums in fp32).

v2 design (vs baseline v1 at 1.23 ms):
  - bf16 matmul operands (fp32r streamed at ~2 cyc/row + 394 ns LDWEIGHTS;
    bf16 streams 1 cyc/row and gets FWL weight loads). PSUM accum stays fp32.
  - host prebuilds the (kh,kd,ci)-duplicated im2col layout xcol
    [D, FRAMES, 73, 1680] bf16 (row 72 = ones for the bias trick), so input
    is ONE dma per d-slice instead of 16 strided ones.
  - j-pairs accumulate directly in PSUM via start/stop flags (j=0,1 -> psA;
    j=2,3 -> psB), eliminating the DVE adds of v1.
  - eviction PSUM->SBUF casts to bf16 (vector engine for pair A, scalar/ACT
    for pair B), halving output DMA; host upcasts and combines.

Device dataflow per core, per output depth slice d (0..39):
  - 1 DMA: xcol[d] [4, 73, 1680] -> xr tile [73, 4*1680] (partition p =
    kh*24 + kd*8 + ci holds the 40x42 window rows [kh, kh+40) of the padded
    plane xpad[ci, d+kd] for each frame; row 72 = ones).
  - per j-pair: 24 matmuls (2 j x 3 kw x 4 h-chunks, N=400, M=128 = 4 blocks
    of 32 channels) accumulate into one psum tile [128, 4x512]. The kw=1
    matmul includes the 73rd ones-row whose weight row is bias. M-block
    layouts differ by j parity so psum block mb always holds output-frame
    slot mb of the pair's accumulator (even j: block mb = k=2-mb, mb=3 zero;
    odd j: block mb = k=3-mb, mb=0 zero).
  - evict psum -> bf16 staging tile -> DMA to DRAM [128, 40, 1600]
    (outA = o_l 0..3 i.e. o = t0-1..t0+2; outB = o_l 2..5).
"""

import numpy as np
import ml_dtypes

BF16 = ml_dtypes.bfloat16

_STATE: dict = {}

# ---- problem constants --------------------------------------------------
B, CI, T, D, H, W = 2, 8, 16, 40, 40, 40
K, CO = 3, 32
O = 16
NCORES = 8
FRAMES = 4          # frames per core
DP, HP, WP = D + 2, H + 2, W + 2   # padded dims
HW = H * WP          # 40*42 = free size of one (h,w') window
NHC = 4              # h-chunks per d-slice
HCROWS = H // NHC    # 10 rows -> N=400 per matmul


def _build_nc():
    import concourse.mybir as mybir
    from concourse import bacc
    from concourse.tile import TileContext

    f32 = mybir.dt.float32
    bf16 = mybir.dt.bfloat16

    nc = bacc.Bacc(
        "TRN2", target_bir_lowering=False, debug=False, num_devices=NCORES
    )
    xcol = nc.dram_tensor("xcol", [D, FRAMES, 73, HW], bf16, kind="ExternalInput")
    wb = nc.dram_tensor("wb", [73, 768], bf16, kind="ExternalInput")
    outA = nc.dram_tensor("outA", [128, D, H * W], bf16, kind="ExternalOutput")
    outB = nc.dram_tensor("outB", [128, D, H * W], bf16, kind="ExternalOutput")

    with TileContext(nc) as tc:
        with (
            tc.tile_pool(name="const", bufs=1) as pc,
            tc.tile_pool(name="xr", bufs=3) as px,
            tc.tile_pool(name="st", bufs=4) as pa,
            tc.tile_pool(name="ps", bufs=2, space="PSUM") as pp,
        ):
            wbt = pc.tile([73, 768], bf16)
            nc.sync.dma_start(wbt[:, :], wb[:, :])
            for d in range(D):
                xr = px.tile([73, FRAMES * HW], bf16, tag="xr")
                nc.sync.dma_start(
                    xr[:, :].rearrange("p (j f) -> p j f", j=FRAMES),
                    xcol[d, :, :, :].rearrange("j p f -> p j f"),
                )
                xrv = xr[:, :].rearrange("p (j h w) -> p j h w", j=FRAMES, w=WP)
                for pair in range(2):
                    ps = pp.tile([128, 4 * 512], f32, tag="ps")
                    for jj in range(2):
                        j = pair * 2 + jj
                        par = j % 2
                        for kw in range(3):
                            rows = 73 if kw == 1 else 72
                            lhsT = wbt[
                                0:rows, (par * 3 + kw) * 128 : (par * 3 + kw + 1) * 128
                            ]
                            for hc in range(NHC):
                                rhs = xrv[
                                    0:rows,
                                    j,
                                    hc * HCROWS : (hc + 1) * HCROWS,
                                    kw : kw + W,
                                ]
                                nc.tensor.matmul(
                                    ps[:, hc * 512 : hc * 512 + HCROWS * W],
                                    lhsT,
                                    rhs,
                                    start=(jj == 0 and kw == 0),
                                    stop=(jj == 1 and kw == 2),
                                )
                    st = pa.tile([128, H * W], bf16, tag="st")
                    psv = ps[:, :].rearrange("p (b c) -> p b c", c=512)[
                        :, :, 0 : HCROWS * W
                    ]
                    stv = st[:, :].rearrange("p (b c) -> p b c", c=HCROWS * W)
                    if pair == 0:
                        nc.vector.tensor_copy(stv, psv)
                        nc.gpsimd.dma_start(outA[:, d, :], st[:, :])
                    else:
                        nc.scalar.activation(
                            stv, psv, mybir.ActivationFunctionType.Copy
                        )
                        nc.gpsimd.dma_start(outB[:, d, :], st[:, :])
    nc.compile()
    return nc


def _get_nc():
    if "nc" not in _STATE:
        _STATE["nc"] = _build_nc()
    return _STATE["nc"]


def _host_inputs(x, weight, bias):
    """Build per-core input maps."""
    x = np.ascontiguousarray(x, dtype=np.float32)
    weight = np.ascontiguousarray(weight, dtype=np.float32)
    bias = np.ascontiguousarray(bias, dtype=np.float32)

    # weight [k, co, ci, kd, kh, kw] -> [kh, kd, ci, kw, k'(=2-k), co]
    wrev = weight.transpose(4, 3, 2, 5, 0, 1)[:, :, :, :, ::-1, :]
    # col layout: par*384 + kw*128 + mb*32 + co
    #   par=0 (even j): blocks 0..2 = k reversed, block 3 zero
    #   par=1 (odd  j): block 0 zero, blocks 1..3 = k reversed
    wbh = np.zeros((73, 768), np.float32)
    w_even = np.zeros((3, 3, 8, 3, 4, 32), np.float32)
    w_even[:, :, :, :, 0:3] = wrev
    w_odd = np.zeros((3, 3, 8, 3, 4, 32), np.float32)
    w_odd[:, :, :, :, 1:4] = wrev
    wbh[0:72, 0:384] = w_even.reshape(72, 384)
    wbh[0:72, 384:768] = w_odd.reshape(72, 384)
    brev = bias[::-1].reshape(96)  # bias in kw=1 block, k-reversed
    wbh[72, 128 : 128 + 96] = brev          # even layout, kw=1, blocks 0..2
    wbh[72, 384 + 128 + 32 : 384 + 256] = brev  # odd layout, kw=1, blocks 1..3
    wbh = wbh.astype(BF16)

    # padded x, then per-core im2col [D, FRAMES, 73, HW] bf16:
    #   xcol[d, j, kh*24+kd*8+ci, h*42+w'] = xpad[b, ci, t0+j, d+kd, kh+h, w']
    xpad = np.zeros((B, CI, T, DP, HP, WP), np.float32)
    xpad[:, :, :, 1 : 1 + D, 1 : 1 + H, 1 : 1 + W] = x

    in_maps = []
    for c in range(NCORES):
        b, tb = divmod(c, 4)
        t0 = tb * FRAMES
        xc = xpad[b, :, t0 : t0 + FRAMES]  # [CI, FRAMES, DP, HP, WP]
        s = xc.strides
        v = np.lib.stride_tricks.as_strided(
            xc,
            shape=(D, FRAMES, 3, 3, CI, H, WP),
            strides=(s[2], s[1], s[3], s[2], s[0], s[3], s[4]),
        )
        xcol = np.empty((D, FRAMES, 73, HW), BF16)
        xcol[:, :, 0:72, :] = v.reshape(D, FRAMES, 72, HW)
        xcol[:, :, 72, :] = 1
        in_maps.append({"xcol": xcol, "wb": wbh})
    return in_maps


def _assemble(results):
    out = np.zeros((B, CO, O, D, H, W), np.float32)
    for c in range(NCORES):
        b, tb = divmod(c, 4)
        t0 = tb * FRAMES
        A = results[c]["outA"].astype(np.float32).reshape(4, 32, D, H, W)
        Bv = results[c]["outB"].astype(np.float32).reshape(4, 32, D, H, W)
        for i in range(4):
            o = t0 - 1 + i
            if 0 <= o < O:
                out[b, :, o] += A[i]
            o = t0 + 1 + i
            if 0 <= o < O:
                out[b, :, o] += Bv[i]
    return out


def _run(x, weight, bias, trace=False, tmpdir=None):
    from concourse.bass_utils import run_bass_kernel_spmd

    if trace:
        _install_ntff_hook()
    nc = _get_nc()
    in_maps = _host_inputs(x, weight, bias)
    res = run_bass_kernel_spmd(
        nc,
        in_maps,
        core_ids=list(range(NCORES)),
        trace=trace,
        tmpdir=tmpdir,
    )
    return _assemble(res.results), res.exec_time_ns


def _install_ntff_hook():
    """Register the axon NTFF profile hook (missing from this image's antenv)."""
    import sys, types

    if "antenv.axon_hooks" in sys.modules:
        return
    mod = types.ModuleType("antenv.axon_hooks")
    holder = [None]
    mod.set_axon_ntff_profile_hook = lambda h: holder.__setitem__(0, h)
    mod.get_axon_ntff_profile_hook = lambda: holder[0]
    sys.modules["antenv.axon_hooks"] = mod
    try:
        from trn_agent_boot.trn_boot import _ntff_profile_via_ctypes

        mod.set_axon_ntff_profile_hook(
            _ntff_profile_via_ctypes("/opt/axon/libaxon_pjrt.so")
        )
    except Exception:
        pass


def kernel(x, weight, bias):
    out, _ = _run(x, weight, bias, trace=False)
    return out


# revision 6
# speedup vs baseline: 1.8086x; 1.8086x over previous
"""Trainium2 Bass kernel for nn_Conv4d (K separate Conv3d layers folded into a
single conv3d with K*Co output channels + temporal accumulation).

Problem (hardcoded):
  x:      [B=2, Ci=8, T=16, D=40, H=40, W=40] f32
  weight: [K=3, Co=32, Ci=8, 3, 3, 3] f32
  bias:   [K=3, Co=32] f32
  out:    [B=2, Co=32, O=16, 40, 40, 40] f32
  out[b, co, o] = sum_k ( conv3d(x[b, :, o+k-1], weight[k], pad=1) + bias[k] )
  with out-of-range temporal frames skipped (zero contribution, incl. bias).

Sharding: data-parallel over the 32 B*T frames, 4 consecutive frames per core.
Each core computes conv3d y[j] = W * x[j] + bias for its 4 frames and
accumulates frame j's three k-blocks into output-frame partials
o = j-1, j, j+1 on-chip; partial sums are combined on the host (adjacent
cores/accumulators overlap at boundaries; host sums in fp32).

v2 design (vs baseline v1 at 1.23 ms):
  - bf16 matmul operands (fp32r streamed at ~2 cyc/row + 394 ns LDWEIGHTS;
    bf16 streams 1 cyc/row and gets FWL weight loads). PSUM accum stays fp32.
  - host prebuilds the (kh,kd,ci)-duplicated im2col layout xcol
    [D, FRAMES, 73, 1680] bf16 (row 72 = ones for the bias trick), so input
    is ONE dma per d-slice instead of 16 strided ones.
  - j-pairs accumulate directly in PSUM via start/stop flags (j=0,1 -> psA;
    j=2,3 -> psB), eliminating the DVE adds of v1.
  - eviction PSUM->SBUF casts to bf16 (vector engine for pair A, scalar/ACT
    for pair B), halving output DMA; host upcasts and combines.

Device dataflow per core, per output depth slice d (0..39):
  - 1 DMA: xcol[d] [4, 73, 1680] -> xr tile [73, 4*1680] (partition p =
    kh*24 + kd*8 + ci holds the 40x42 window rows [kh, kh+40) of the padded
    plane xpad[ci, d+kd] for each frame; row 72 = ones).
  - per j-pair: 24 matmuls (2 j x 3 kw x 4 h-chunks, N=400, M=128 = 4 blocks
    of 32 channels) accumulate into one psum tile [128, 4x512]. The kw=1
    matmul includes the 73rd ones-row whose weight row is bias. M-block
    layouts differ by j parity so psum block mb always holds output-frame
    slot mb of the pair's accumulator (even j: block mb = k=2-mb, mb=3 zero;
    odd j: block mb = k=3-mb, mb=0 zero).
  - evict psum -> bf16 staging tile -> DMA to DRAM [128, 40, 1600]
    (outA = o_l 0..3 i.e. o = t0-1..t0+2; outB = o_l 2..5).
"""

import numpy as np
import ml_dtypes

BF16 = ml_dtypes.bfloat16

_STATE: dict = {}

# ---- problem constants --------------------------------------------------
B, CI, T, D, H, W = 2, 8, 16, 40, 40, 40
K, CO = 3, 32
O = 16
NCORES = 8
FRAMES = 4          # frames per core
DP, HP, WP = D + 2, H + 2, W + 2   # padded dims
HW = H * WP          # 40*42 = free size of one (h,w') window
NHC = 4              # h-chunks per d-slice
HCROWS = H // NHC    # 10 rows -> N=400 per matmul


def _build_nc():
    import concourse.mybir as mybir
    from concourse import bacc
    from concourse.tile import TileContext

    f32 = mybir.dt.float32
    bf16 = mybir.dt.bfloat16

    nc = bacc.Bacc(
        "TRN2", target_bir_lowering=False, debug=False, num_devices=NCORES
    )
    xcol = nc.dram_tensor("xcol", [D, 73, FRAMES * HW], bf16, kind="ExternalInput")
    wb = nc.dram_tensor("wb", [73, 768], bf16, kind="ExternalInput")
    outA = nc.dram_tensor("outA", [128, D, H * W], bf16, kind="ExternalOutput")
    outB = nc.dram_tensor("outB", [128, D, H * W], bf16, kind="ExternalOutput")

    with TileContext(nc) as tc:
        with (
            tc.tile_pool(name="const", bufs=1) as pc,
            tc.tile_pool(name="xr", bufs=3) as px,
            tc.tile_pool(name="st", bufs=4) as pa,
            tc.tile_pool(name="ps", bufs=2, space="PSUM") as pp,
        ):
            wbt = pc.tile([73, 768], bf16)
            nc.sync.dma_start(wbt[:, :], wb[:, :])
            for d in range(D):
                xr = px.tile([73, FRAMES * HW], bf16, tag="xr")
                nc.gpsimd.dma_start(xr[:, :], xcol[d, :, :])
                xrv = xr[:, :].rearrange("p (j h w) -> p j h w", j=FRAMES, w=WP)
                for pair in range(2):
                    ps = pp.tile([128, 4 * 512], f32, tag="ps")
                    for jj in range(2):
                        j = pair * 2 + jj
                        par = j % 2
                        for kw in range(3):
                            rows = 73 if kw == 1 else 72
                            lhsT = wbt[
                                0:rows, (par * 3 + kw) * 128 : (par * 3 + kw + 1) * 128
                            ]
                            for hc in range(NHC):
                                rhs = xrv[
                                    0:rows,
                                    j,
                                    hc * HCROWS : (hc + 1) * HCROWS,
                                    kw : kw + W,
                                ]
                                nc.tensor.matmul(
                                    ps[:, hc * 512 : hc * 512 + HCROWS * W],
                                    lhsT,
                                    rhs,
                                    start=(jj == 0 and kw == 0),
                                    stop=(jj == 1 and kw == 2),
                                )
                    st = pa.tile([128, H * W], bf16, tag="st")
                    psv = ps[:, :].rearrange("p (b c) -> p b c", c=512)[
                        :, :, 0 : HCROWS * W
                    ]
                    stv = st[:, :].rearrange("p (b c) -> p b c", c=HCROWS * W)
                    if pair == 0:
                        nc.vector.tensor_copy(stv, psv)
                        nc.gpsimd.dma_start(outA[:, d, :], st[:, :])
                    else:
                        nc.scalar.activation(
                            stv, psv, mybir.ActivationFunctionType.Copy
                        )
                        nc.gpsimd.dma_start(outB[:, d, :], st[:, :])
    nc.compile()
    return nc


def _get_nc():
    if "nc" not in _STATE:
        _STATE["nc"] = _build_nc()
    return _STATE["nc"]


def _host_inputs(x, weight, bias):
    """Build per-core input maps."""
    x = np.ascontiguousarray(x, dtype=np.float32)
    weight = np.ascontiguousarray(weight, dtype=np.float32)
    bias = np.ascontiguousarray(bias, dtype=np.float32)

    # weight [k, co, ci, kd, kh, kw] -> [kh, kd, ci, kw, k'(=2-k), co]
    wrev = weight.transpose(4, 3, 2, 5, 0, 1)[:, :, :, :, ::-1, :]
    # col layout: par*384 + kw*128 + mb*32 + co
    #   par=0 (even j): blocks 0..2 = k reversed, block 3 zero
    #   par=1 (odd  j): block 0 zero, blocks 1..3 = k reversed
    wbh = np.zeros((73, 768), np.float32)
    w_even = np.zeros((3, 3, 8, 3, 4, 32), np.float32)
    w_even[:, :, :, :, 0:3] = wrev
    w_odd = np.zeros((3, 3, 8, 3, 4, 32), np.float32)
    w_odd[:, :, :, :, 1:4] = wrev
    wbh[0:72, 0:384] = w_even.reshape(72, 384)
    wbh[0:72, 384:768] = w_odd.reshape(72, 384)
    brev = bias[::-1].reshape(96)  # bias in kw=1 block, k-reversed
    wbh[72, 128 : 128 + 96] = brev          # even layout, kw=1, blocks 0..2
    wbh[72, 384 + 128 + 32 : 384 + 256] = brev  # odd layout, kw=1, blocks 1..3
    wbh = wbh.astype(BF16)

    # padded x, then per-core im2col [D, FRAMES, 73, HW] bf16:
    #   xcol[d, j, kh*24+kd*8+ci, h*42+w'] = xpad[b, ci, t0+j, d+kd, kh+h, w']
    xpad = np.zeros((B, CI, T, DP, HP, WP), np.float32)
    xpad[:, :, :, 1 : 1 + D, 1 : 1 + H, 1 : 1 + W] = x

    in_maps = []
    for c in range(NCORES):
        b, tb = divmod(c, 4)
        t0 = tb * FRAMES
        xc = xpad[b, :, t0 : t0 + FRAMES]  # [CI, FRAMES, DP, HP, WP]
        s = xc.strides
        v = np.lib.stride_tricks.as_strided(
            xc,
            shape=(D, FRAMES, 3, 3, CI, H, WP),
            strides=(s[2], s[1], s[3], s[2], s[0], s[3], s[4]),
        )
        xcol = np.empty((D, 73, FRAMES, HW), BF16)
        xcol[:, 0:72] = v.reshape(D, FRAMES, 72, HW).transpose(0, 2, 1, 3)
        xcol[:, 72] = 1
        xcol = xcol.reshape(D, 73, FRAMES * HW)
        in_maps.append({"xcol": xcol, "wb": wbh})
    return in_maps


def _assemble(results):
    out = np.zeros((B, CO, O, D, H, W), np.float32)
    for c in range(NCORES):
        b, tb = divmod(c, 4)
        t0 = tb * FRAMES
        A = results[c]["outA"].astype(np.float32).reshape(4, 32, D, H, W)
        Bv = results[c]["outB"].astype(np.float32).reshape(4, 32, D, H, W)
        for i in range(4):
            o = t0 - 1 + i
            if 0 <= o < O:
                out[b, :, o] += A[i]
            o = t0 + 1 + i
            if 0 <= o < O:
                out[b, :, o] += Bv[i]
    return out


def _run(x, weight, bias, trace=False, tmpdir=None):
    from concourse.bass_utils import run_bass_kernel_spmd

    if trace:
        _install_ntff_hook()
    nc = _get_nc()
    in_maps = _host_inputs(x, weight, bias)
    res = run_bass_kernel_spmd(
        nc,
        in_maps,
        core_ids=list(range(NCORES)),
        trace=trace,
        tmpdir=tmpdir,
    )
    return _assemble(res.results), res.exec_time_ns


def _install_ntff_hook():
    """Register the axon NTFF profile hook (missing from this image's antenv)."""
    import sys, types

    if "antenv.axon_hooks" in sys.modules:
        return
    mod = types.ModuleType("antenv.axon_hooks")
    holder = [None]
    mod.set_axon_ntff_profile_hook = lambda h: holder.__setitem__(0, h)
    mod.get_axon_ntff_profile_hook = lambda: holder[0]
    sys.modules["antenv.axon_hooks"] = mod
    try:
        from trn_agent_boot.trn_boot import _ntff_profile_via_ctypes

        mod.set_axon_ntff_profile_hook(
            _ntff_profile_via_ctypes("/opt/axon/libaxon_pjrt.so")
        )
    except Exception:
        pass


def kernel(x, weight, bias):
    out, _ = _run(x, weight, bias, trace=False)
    return out


# revision 9
# speedup vs baseline: 1.8567x; 1.0266x over previous
"""Trainium2 Bass kernel for nn_Conv4d (K separate Conv3d layers folded into a
single conv3d with K*Co output channels + temporal accumulation).

Problem (hardcoded):
  x:      [B=2, Ci=8, T=16, D=40, H=40, W=40] f32
  weight: [K=3, Co=32, Ci=8, 3, 3, 3] f32
  bias:   [K=3, Co=32] f32
  out:    [B=2, Co=32, O=16, 40, 40, 40] f32
  out[b, co, o] = sum_k ( conv3d(x[b, :, o+k-1], weight[k], pad=1) + bias[k] )
  with out-of-range temporal frames skipped (zero contribution, incl. bias).

Sharding: data-parallel over the 32 B*T frames, 4 consecutive frames per core.
Each core computes conv3d y[j] = W * x[j] + bias for its 4 frames and
accumulates frame j's three k-blocks into output-frame partials
o = j-1, j, j+1 on-chip; partial sums are combined on the host (adjacent
cores/accumulators overlap at boundaries; host sums in fp32).

v2 design (vs baseline v1 at 1.23 ms):
  - bf16 matmul operands (fp32r streamed at ~2 cyc/row + 394 ns LDWEIGHTS;
    bf16 streams 1 cyc/row and gets FWL weight loads). PSUM accum stays fp32.
  - host prebuilds the (kh,kd,ci)-duplicated im2col layout xcol
    [D, FRAMES, 73, 1680] bf16 (row 72 = ones for the bias trick), so input
    is ONE dma per d-slice instead of 16 strided ones.
  - j-pairs accumulate directly in PSUM via start/stop flags (j=0,1 -> psA;
    j=2,3 -> psB), eliminating the DVE adds of v1.
  - eviction PSUM->SBUF casts to bf16 (vector engine for pair A, scalar/ACT
    for pair B), halving output DMA; host upcasts and combines.

Device dataflow per core, per output depth slice d (0..39):
  - 1 DMA: xcol[d] [4, 73, 1680] -> xr tile [73, 4*1680] (partition p =
    kh*24 + kd*8 + ci holds the 40x42 window rows [kh, kh+40) of the padded
    plane xpad[ci, d+kd] for each frame; row 72 = ones).
  - per j-pair: 24 matmuls (2 j x 3 kw x 4 h-chunks, N=400, M=128 = 4 blocks
    of 32 channels) accumulate into one psum tile [128, 4x512]. The kw=1
    matmul includes the 73rd ones-row whose weight row is bias. M-block
    layouts differ by j parity so psum block mb always holds output-frame
    slot mb of the pair's accumulator (even j: block mb = k=2-mb, mb=3 zero;
    odd j: block mb = k=3-mb, mb=0 zero).
  - evict psum -> bf16 staging tile -> DMA to DRAM [128, 40, 1600]
    (outA = o_l 0..3 i.e. o = t0-1..t0+2; outB = o_l 2..5).
"""

import numpy as np
import ml_dtypes

BF16 = ml_dtypes.bfloat16

_STATE: dict = {}

# ---- problem constants --------------------------------------------------
B, CI, T, D, H, W = 2, 8, 16, 40, 40, 40
K, CO = 3, 32
O = 16
NCORES = 8
FRAMES = 4          # frames per core
DP, HP, WP = D + 2, H + 2, W + 2   # padded dims
HW = H * WP          # 40*42 = free size of one (h,w') window
NHC = 4              # h-chunks per d-slice
HCROWS = H // NHC    # 10 rows -> N=400 per matmul


def _build_nc():
    import concourse.mybir as mybir
    from concourse import bacc
    from concourse.tile import TileContext

    f32 = mybir.dt.float32
    bf16 = mybir.dt.bfloat16

    nc = bacc.Bacc(
        "TRN2", target_bir_lowering=False, debug=False, num_devices=NCORES
    )
    xcol = nc.dram_tensor("xcol", [D, FRAMES, 73, HW], bf16, kind="ExternalInput")
    wb = nc.dram_tensor("wb", [73, 768], bf16, kind="ExternalInput")
    outA = nc.dram_tensor("outA", [128, D, H * W], bf16, kind="ExternalOutput")
    outB = nc.dram_tensor("outB", [128, D, H * W], bf16, kind="ExternalOutput")

    with TileContext(nc) as tc:
        with (
            tc.tile_pool(name="const", bufs=1) as pc,
            tc.tile_pool(name="xr", bufs=3) as px,
            tc.tile_pool(name="st", bufs=4) as pa,
            tc.tile_pool(name="ps", bufs=2, space="PSUM") as pp,
        ):
            wbt = pc.tile([73, 768], bf16)
            nc.sync.dma_start(wbt[:, :], wb[:, :])
            for d in range(D):
                xr = px.tile([73, FRAMES * HW], bf16, tag="xr")
                nc.gpsimd.dma_start(
                    xr[:, :].rearrange("p (j f) -> p j f", j=FRAMES),
                    xcol[d, :, :, :].rearrange("j p f -> p j f"),
                )
                xrv = xr[:, :].rearrange("p (j h w) -> p j h w", j=FRAMES, w=WP)
                for pair in range(2):
                    ps = pp.tile([128, 4 * 512], f32, tag="ps")
                    for jj in range(2):
                        j = pair * 2 + jj
                        par = j % 2
                        for kw in range(3):
                            rows = 73 if kw == 1 else 72
                            lhsT = wbt[
                                0:rows, (par * 3 + kw) * 128 : (par * 3 + kw + 1) * 128
                            ]
                            for hc in range(NHC):
                                rhs = xrv[
                                    0:rows,
                                    j,
                                    hc * HCROWS : (hc + 1) * HCROWS,
                                    kw : kw + W,
                                ]
                                nc.tensor.matmul(
                                    ps[:, hc * 512 : hc * 512 + HCROWS * W],
                                    lhsT,
                                    rhs,
                                    start=(jj == 0 and kw == 0),
                                    stop=(jj == 1 and kw == 2),
                                )
                    st = pa.tile([128, H * W], bf16, tag="st")
                    psv = ps[:, :].rearrange("p (b c) -> p b c", c=512)[
                        :, :, 0 : HCROWS * W
                    ]
                    stv = st[:, :].rearrange("p (b c) -> p b c", c=HCROWS * W)
                    if pair == 0:
                        nc.vector.tensor_copy(stv, psv)
                        nc.gpsimd.dma_start(outA[:, d, :], st[:, :])
                    else:
                        nc.scalar.activation(
                            stv, psv, mybir.ActivationFunctionType.Copy
                        )
                        nc.gpsimd.dma_start(outB[:, d, :], st[:, :])
    nc.compile()
    return nc


def _get_nc():
    if "nc" not in _STATE:
        _STATE["nc"] = _build_nc()
    return _STATE["nc"]


def _host_inputs(x, weight, bias):
    """Build per-core input maps."""
    x = np.ascontiguousarray(x, dtype=np.float32)
    weight = np.ascontiguousarray(weight, dtype=np.float32)
    bias = np.ascontiguousarray(bias, dtype=np.float32)

    # weight [k, co, ci, kd, kh, kw] -> [kh, kd, ci, kw, k'(=2-k), co]
    wrev = weight.transpose(4, 3, 2, 5, 0, 1)[:, :, :, :, ::-1, :]
    # col layout: par*384 + kw*128 + mb*32 + co
    #   par=0 (even j): blocks 0..2 = k reversed, block 3 zero
    #   par=1 (odd  j): block 0 zero, blocks 1..3 = k reversed
    wbh = np.zeros((73, 768), np.float32)
    w_even = np.zeros((3, 3, 8, 3, 4, 32), np.float32)
    w_even[:, :, :, :, 0:3] = wrev
    w_odd = np.zeros((3, 3, 8, 3, 4, 32), np.float32)
    w_odd[:, :, :, :, 1:4] = wrev
    wbh[0:72, 0:384] = w_even.reshape(72, 384)
    wbh[0:72, 384:768] = w_odd.reshape(72, 384)
    brev = bias[::-1].reshape(96)  # bias in kw=1 block, k-reversed
    wbh[72, 128 : 128 + 96] = brev          # even layout, kw=1, blocks 0..2
    wbh[72, 384 + 128 + 32 : 384 + 256] = brev  # odd layout, kw=1, blocks 1..3
    wbh = wbh.astype(BF16)

    # padded x, then per-core im2col [D, FRAMES, 73, HW] bf16:
    #   xcol[d, j, kh*24+kd*8+ci, h*42+w'] = xpad[b, ci, t0+j, d+kd, kh+h, w']
    xpad = np.zeros((B, CI, T, DP, HP, WP), np.float32)
    xpad[:, :, :, 1 : 1 + D, 1 : 1 + H, 1 : 1 + W] = x

    in_maps = []
    for c in range(NCORES):
        b, tb = divmod(c, 4)
        t0 = tb * FRAMES
        xc = xpad[b, :, t0 : t0 + FRAMES]  # [CI, FRAMES, DP, HP, WP]
        s = xc.strides
        v = np.lib.stride_tricks.as_strided(
            xc,
            shape=(D, FRAMES, 3, 3, CI, H, WP),
            strides=(s[2], s[1], s[3], s[2], s[0], s[3], s[4]),
        )
        xcol = np.empty((D, FRAMES, 73, HW), BF16)
        xcol[:, :, 0:72, :] = v.reshape(D, FRAMES, 72, HW)
        xcol[:, :, 72, :] = 1
        in_maps.append({"xcol": xcol, "wb": wbh})
    return in_maps


def _assemble(results):
    out = np.zeros((B, CO, O, D, H, W), np.float32)
    for c in range(NCORES):
        b, tb = divmod(c, 4)
        t0 = tb * FRAMES
        A = results[c]["outA"].astype(np.float32).reshape(4, 32, D, H, W)
        Bv = results[c]["outB"].astype(np.float32).reshape(4, 32, D, H, W)
        for i in range(4):
            o = t0 - 1 + i
            if 0 <= o < O:
                out[b, :, o] += A[i]
            o = t0 + 1 + i
            if 0 <= o < O:
                out[b, :, o] += Bv[i]
    return out


def _run(x, weight, bias, trace=False, tmpdir=None):
    from concourse.bass_utils import run_bass_kernel_spmd

    if trace:
        _install_ntff_hook()
    nc = _get_nc()
    in_maps = _host_inputs(x, weight, bias)
    res = run_bass_kernel_spmd(
        nc,
        in_maps,
        core_ids=list(range(NCORES)),
        trace=trace,
        tmpdir=tmpdir,
    )
    return _assemble(res.results), res.exec_time_ns


def _install_ntff_hook():
    """Register the axon NTFF profile hook (missing from this image's antenv)."""
    import sys, types

    if "antenv.axon_hooks" in sys.modules:
        return
    mod = types.ModuleType("antenv.axon_hooks")
    holder = [None]
    mod.set_axon_ntff_profile_hook = lambda h: holder.__setitem__(0, h)
    mod.get_axon_ntff_profile_hook = lambda: holder[0]
    sys.modules["antenv.axon_hooks"] = mod
    try:
        from trn_agent_boot.trn_boot import _ntff_profile_via_ctypes

        mod.set_axon_ntff_profile_hook(
            _ntff_profile_via_ctypes("/opt/axon/libaxon_pjrt.so")
        )
    except Exception:
        pass


def kernel(x, weight, bias):
    out, _ = _run(x, weight, bias, trace=False)
    return out
